# revision 6
# baseline (speedup 1.0000x reference)
"""GroupedQueryAttention on 8 Trainium2 NeuronCores — v3.

Problem (hardcoded): B=2, T=2048, DIM=4096, 32 q heads, 8 kv heads, hd=128.
  q = x @ Wq.T ; k,v = split(x @ Wkv.T) ; causal softmax(q k^T/sqrt(hd)) v ; out = o @ Wo.T

Sharding: hybrid data x tensor parallel over 8 cores.
  core c -> batch b = c//4, kv-head group j = c%4 (kv heads {2j,2j+1}, q heads {8j..8j+7}).

v3 changes over v2 (1.063 ms):
  P1: xT held resident in SBUF (one 16.8 MB load) — matmuls never wait on
      activations; Wall weights streamed in [128,512] pieces per (chunk, cb).
  P2: the softmax reciprocal moved off the vector FIFO (the v2 convoy):
      1/den = Exp(-Ln(den)) on the scalar engine; the whole per-(h,tg) tail
      (den matmuls, recip, broadcast, normalize) is emitted one group LATE so
      no engine FIFO head-blocks on it; AV matmuls lag the score matmuls by
      two pairs so the tensor queue never waits on exp.
  P3: unchanged from v2 (runs at the per-matmul floor with zero big gaps).
"""

import sys

sys.path.insert(0, "/opt/trn_rl_repo")

import math

import numpy as np

import concourse.bass as bass
import concourse.bacc as bacc
import concourse.tile as tile
from concourse import mybir
from concourse.bass_utils import run_bass_kernel_spmd

B, T, DIM = 2, 2048, 4096
N_HEADS, N_KV, HD = 32, 8, 128
R = N_HEADS // N_KV  # 4
NCORES = 8
GROUPS = [[0, 1, 2, 3], [4, 5, 6, 7]]

HPC = 8  # q heads per core
KVPC = 2  # kv heads per core
EQ = HPC * HD  # 1024 q-proj out features per core
EKV = KVPC * HD  # 256 k (and v) out features per core
NT = T // 512  # 4 t-groups of 512
NC = DIM // 128  # 32 contraction tiles
NKB = T // 128  # 16 k-tiles per head

BF = mybir.dt.bfloat16
F32 = mybir.dt.float32
INV_SQRT_HD = 1.0 / math.sqrt(HD)


def build():
    nc = bacc.Bacc("TRN2", num_devices=NCORES)

    # ---- external I/O (per-core data differs, program is SPMD-identical) ----
    xT = nc.dram_tensor("xT", [DIM, T], BF, kind="ExternalInput")  # x[b].T
    wallT = nc.dram_tensor("wallT", [DIM, EQ + 2 * EKV], BF, kind="ExternalInput")
    woT = nc.dram_tensor("woT", [DIM, EQ], BF, kind="ExternalInput")  # Wo[oc_slice,:].T
    maskA = nc.dram_tensor("maskA", [128, 1024], BF, kind="ExternalInput")
    maskB = nc.dram_tensor("maskB", [128, 1024], BF, kind="ExternalInput")
    ident = nc.dram_tensor("ident", [128, 128], BF, kind="ExternalInput")
    ones_in = nc.dram_tensor("ones_in", [128, 1], BF, kind="ExternalInput")
    out_part = nc.dram_tensor("out_part", [EQ, T], F32, kind="ExternalOutput")

    EALL = EQ + 2 * EKV  # 1536, 12 e-tiles: 8 Q, 2 K, 2 V
    # wallT column offset of each chunk's 4 e-tiles
    chunks = [[8, 9, 10, 11], [0, 1, 2, 3], [4, 5, 6, 7]]
    chunk_c0 = [1024, 0, 512]

    with tile.TileContext(nc) as tc:
        with (
            tc.tile_pool(name="persist", bufs=1) as persist,
            tc.tile_pool(name="work", bufs=3) as work,
            tc.tile_pool(name="dram2", bufs=1, space="DRAM") as dram2,
        ):
            # ---------------- constants ----------------
            maskA_sb = persist.tile([128, 1024], BF)
            nc.sync.dma_start(out=maskA_sb[:], in_=maskA[:, :])
            maskB_sb = persist.tile([128, 1024], BF)
            nc.sync.dma_start(out=maskB_sb[:], in_=maskB[:, :])
            ident_sb = persist.tile([128, 128], BF)
            nc.sync.dma_start(out=ident_sb[:], in_=ident[:, :])
            ones_sb = persist.tile([128, 1], BF)
            nc.sync.dma_start(out=ones_sb[:], in_=ones_in[:, :])

            # persistent activations
            qt_sb = persist.tile([128, HPC * T], BF)  # QT: head h at cols [h*T,(h+1)*T)
            kt_sb = persist.tile([128, KVPC * T], BF)  # KT per kv head
            v_sb = persist.tile([128, KVPC * T], BF)  # V[t,dv]: tile (g,kb) at (g*16+kb)*128

            # per-head AllGather buffers (head 7 split in halves)
            og_in = []
            og_out = []
            for h in range(HPC - 1):
                og_in.append(dram2.tile([128, T], BF, name=f"og_in_{h}"))
                og_out.append(dram2.tile([4 * 128, T], BF, name=f"og_out_{h}"))
            og_in7 = [dram2.tile([128, 1024], BF, name=f"og_in7_{i}",
                                 tag=f"og_in7_{i}") for i in range(2)]
            og_out7 = [dram2.tile([4 * 128, 1024], BF, name=f"og_out7_{i}",
                                  tag=f"og_out7_{i}") for i in range(2)]

            with (
                tc.tile_pool(name="p1pool", bufs=1) as p1pool,
                tc.tile_pool(name="wpiece", bufs=6) as wpiece,
                tc.tile_pool(name="psum_p1", bufs=1, space="PSUM") as psum_p1,
            ):
                # resident activations: strip cb at cols [cb*T, (cb+1)*T)
                xT_sb = p1pool.tile([128, NC * T], BF)
                for cb in range(NC):
                    nc.sync.dma_start(
                        out=xT_sb[:, cb * T:(cb + 1) * T],
                        in_=xT[cb * 128:(cb + 1) * 128, :],
                    )
                vt_sb = p1pool.tile([128, KVPC * T], BF)  # VT per kv head (P1 only)

                # ---------------- phase 1: projections ----------------
                def etile_dst(e):
                    # e indexes [Q0..Q7, K0, K1, V0, V1]
                    if e < HPC:
                        return qt_sb[:, e * T:(e + 1) * T]
                    if e < HPC + KVPC:
                        g = e - HPC
                        return kt_sb[:, g * T:(g + 1) * T]
                    g = e - HPC - KVPC
                    return vt_sb[:, g * T:(g + 1) * T]

                for ci, es in enumerate(chunks):
                    c0 = chunk_c0[ci]
                    for tgp in range(2):  # t-group pairs {0,1}, {2,3}
                        accs = [[psum_p1.tile([128, 512], F32, tag=f"acc{i}{j}",
                                              name=f"acc{i}{j}")
                                 for j in range(2)] for i in range(4)]
                        for cb in range(NC):
                            wp = wpiece.tile([128, 512], BF, tag="wp")
                            nc.sync.dma_start(
                                out=wp[:],
                                in_=wallT[cb * 128:(cb + 1) * 128, c0:c0 + 512],
                            )
                            for i in range(4):
                                w = wp[:, i * 128:(i + 1) * 128]
                                for j in range(2):
                                    t0 = (2 * tgp + j) * 512
                                    nc.tensor.matmul(
                                        accs[i][j][:], w,
                                        xT_sb[:, cb * T + t0:cb * T + t0 + 512],
                                        start=(cb == 0), stop=(cb == NC - 1),
                                    )
                        for i, e in enumerate(es):
                            for j in range(2):
                                t0 = (2 * tgp + j) * 512
                                nc.vector.tensor_copy(
                                    etile_dst(e)[:, t0:t0 + 512], accs[i][j][:]
                                )
                    if ci == 0:
                        # V = VT.T per 128x128 tile (PE transpose-mode)
                        for g in range(KVPC):
                            for kb in range(NKB):
                                tp = psum_p1.tile([128, 128], BF,
                                                  tag=f"acc{kb % 4}{g}")
                                nc.tensor.transpose(
                                    tp[:],
                                    vt_sb[:, g * T + kb * 128:
                                          g * T + (kb + 1) * 128],
                                    ident_sb[:],
                                )
                                nc.vector.tensor_copy(
                                    v_sb[:, (g * NKB + kb) * 128:
                                         (g * NKB + kb + 1) * 128],
                                    tp[:],
                                )

            # p1pool/psum_p1 released; phase 2/3 reuse that SBUF/PSUM space.
            with (
                tc.tile_pool(name="p23", bufs=1) as p23,
                tc.tile_pool(name="work2", bufs=3) as work2,
            ):
                oT_sb = p23.tile([128, HPC * T], BF)  # local oT: head h at [h*T,..)
                woT_sb = p23.tile([128, NC * EQ], BF)  # phase-3 lhsT tiles
                for cb in range(NC):
                    nc.sync.dma_start(
                        out=woT_sb[:, cb * EQ:(cb + 1) * EQ],
                        in_=woT[cb * 128:(cb + 1) * 128, :],
                    )

                # ---------------- phase 2: attention ----------------
                with (
                    tc.tile_pool(name="ps_sT", bufs=2, space="PSUM") as ps_sT,
                    tc.tile_pool(name="ps_oT", bufs=2, space="PSUM") as ps_oT,
                    tc.tile_pool(name="ps_den", bufs=2, space="PSUM") as ps_den,
                ):
                    pending_tail = [None]

                    def flush_tail():
                        if pending_tail[0] is not None:
                            pending_tail[0]()
                            pending_tail[0] = None

                    for h in range(HPC):
                        g = h // R  # local kv head
                        qt_h = qt_sb[:, h * T:(h + 1) * T]
                        kt_g = kt_sb[:, g * T:(g + 1) * T]
                        for tg in range(NT):
                            npairs = 2 * tg + 2  # k-tile pairs 0..npairs-1
                            qs = qt_h[:, tg * 512:(tg + 1) * 512]
                            oT_acc = ps_oT.tile([128, 512], F32, tag="oT")
                            expsum = work2.tile([128, 1024], BF, tag="expsum",
                                                bufs=2)
                            expps = []

                            def emit_av(p, expp, _oT=oT_acc, _np=npairs, _g=g,
                                        _tg=tg):
                                first = (p == 0)
                                last = (p == _np - 1)
                                for j in range(2):
                                    kb = 2 * p + j
                                    nc.tensor.matmul(
                                        _oT[:],
                                        v_sb[:, (_g * NKB + kb) * 128:
                                             (_g * NKB + kb + 1) * 128],
                                        expp[:, j * 512:(j + 1) * 512],
                                        start=(first and j == 0),
                                        stop=(last and j == 1),
                                        skip_group_check=True,
                                    )

                            for p in range(npairs):
                                sT2 = ps_sT.tile([128, 1024], F32, tag="sT2")
                                for j in range(2):
                                    kb = 2 * p + j
                                    nc.tensor.matmul(
                                        sT2[:, j * 512:(j + 1) * 512],
                                        kt_g[:, kb * 128:(kb + 1) * 128],
                                        qs,
                                        start=True, stop=True,
                                        skip_group_check=True,
                                    )
                                # previous group's tail goes out while this
                                # group's first pair streams (no FIFO blocks)
                                if p == 1:
                                    flush_tail()
                                expp = work2.tile([128, 1024], BF, tag="expT2",
                                                  bufs=4)
                                nc.scalar.activation(
                                    expp[:], sT2[:],
                                    mybir.ActivationFunctionType.Exp,
                                    scale=INV_SQRT_HD,
                                )
                                # diagonal pairs: multiplicative causal mask
                                if p == npairs - 2:
                                    nc.vector.tensor_tensor(
                                        expp[:], expp[:], maskA_sb[:],
                                        mybir.AluOpType.mult,
                                    )
                                elif p == npairs - 1:
                                    nc.vector.tensor_tensor(
                                        expp[:], expp[:], maskB_sb[:],
                                        mybir.AluOpType.mult,
                                    )
                                # expsum accumulate (bf16, [128,1024])
                                if p == 0:
                                    nc.vector.tensor_copy(expsum[:], expp[:])
                                else:
                                    nc.vector.tensor_tensor(
                                        expsum[:], expsum[:], expp[:],
                                        mybir.AluOpType.add,
                                    )
                                # AV lags two pairs behind the score matmuls
                                expps.append(expp)
                                if p >= 2:
                                    emit_av(p - 2, expps[p - 2])
                            for p in (npairs - 2, npairs - 1):
                                emit_av(p, expps[p])

                            def tail(_h=h, _tg=tg, _oT=oT_acc, _es=expsum):
                                den_acc = ps_den.tile([1, 512], F32, tag="den")
                                for j in range(2):
                                    nc.tensor.matmul(
                                        den_acc[:], ones_sb[:],
                                        _es[:, j * 512:(j + 1) * 512],
                                        start=(j == 0), stop=(j == 1),
                                        skip_group_check=True,
                                    )
                                # 1/den = Exp(-Ln(den)) on the scalar engine —
                                # keeps the slow iterative divide off VectorE
                                lnden = work2.tile([1, 512], F32, tag="lnden")
                                nc.scalar.activation(
                                    lnden[:], den_acc[:],
                                    mybir.ActivationFunctionType.Ln,
                                )
                                recip = work2.tile([1, 512], F32, tag="recip")
                                nc.scalar.activation(
                                    recip[:], lnden[:],
                                    mybir.ActivationFunctionType.Exp,
                                    scale=-1.0,
                                )
                                recip_b = work2.tile([128, 512], F32,
                                                     tag="recip_b")
                                nc.gpsimd.partition_broadcast(recip_b[:],
                                                              recip[:])
                                nc.vector.tensor_tensor(
                                    oT_sb[:, _h * T + _tg * 512:
                                          _h * T + (_tg + 1) * 512],
                                    _oT[:],
                                    recip_b[:],
                                    mybir.AluOpType.mult,
                                )
                                # ship completed halves/heads
                                if _h == HPC - 1 and _tg in (1, 3):
                                    i7 = _tg // 2
                                    nc.sync.dma_start(
                                        out=og_in7[i7][:],
                                        in_=oT_sb[:, _h * T + i7 * 1024:
                                                  _h * T + (i7 + 1) * 1024],
                                    )
                                    nc.gpsimd.collective_compute(
                                        "AllGather",
                                        mybir.AluOpType.bypass,
                                        replica_groups=GROUPS,
                                        ins=[og_in7[i7].opt()],
                                        outs=[og_out7[i7].opt()],
                                    )
                                elif _h < HPC - 1 and _tg == 3:
                                    nc.sync.dma_start(
                                        out=og_in[_h][:],
                                        in_=oT_sb[:, _h * T:(_h + 1) * T],
                                    )
                                    nc.gpsimd.collective_compute(
                                        "AllGather",
                                        mybir.AluOpType.bypass,
                                        replica_groups=GROUPS,
                                        ins=[og_in[_h].opt()],
                                        outs=[og_out[_h].opt()],
                                    )

                            flush_tail()  # only fires if still pending (tg==0
                            # of a head whose previous tail wasn't flushed)
                            pending_tail[0] = tail
                        # end tg loop
                    flush_tail()

                # ---------------- phase 3: outT slice = WoT.T @ oT_full --------
                eb_order = [rr * HPC + hh for hh in range(HPC) for rr in range(4)]
                with tc.tile_pool(name="ps_out", bufs=1, space="PSUM") as ps_out:
                    for ocp in range(2):
                        for tgp in range(2):
                            accs = [[ps_out.tile([128, 512], F32,
                                                 tag=f"out{oi}{j}",
                                                 name=f"out{oi}{j}")
                                     for j in range(2)] for oi in range(4)]
                            for ei, eb in enumerate(eb_order):
                                r, hl = eb // HPC, eb % HPC
                                rhs_t = work2.tile([128, 1024], BF, tag="rhs",
                                                   bufs=6)
                                if hl == HPC - 1:
                                    src = og_out7[tgp][r * 128:(r + 1) * 128, :]
                                else:
                                    src = og_out[hl][r * 128:(r + 1) * 128,
                                                     tgp * 1024:(tgp + 1) * 1024]
                                nc.sync.dma_start(out=rhs_t[:], in_=src)
                                for oi in range(4):
                                    oc = ocp * 4 + oi
                                    w = woT_sb[:, eb * EQ + oc * 128:
                                               eb * EQ + (oc + 1) * 128]
                                    for j in range(2):
                                        nc.tensor.matmul(
                                            accs[oi][j][:],
                                            w,
                                            rhs_t[:, j * 512:(j + 1) * 512],
                                            start=(ei == 0),
                                            stop=(ei == NC - 1),
                                        )
                            for oi in range(4):
                                oc = ocp * 4 + oi
                                for j in range(2):
                                    t0 = tgp * 1024 + j * 512
                                    ev = work2.tile([128, 512], F32, tag="ev",
                                                    bufs=4)
                                    nc.vector.tensor_copy(ev[:], accs[oi][j][:])
                                    nc.sync.dma_start(
                                        out=out_part[oc * 128:(oc + 1) * 128,
                                                     t0:t0 + 512],
                                        in_=ev[:],
                                    )
    nc.finalize()
    return nc


_NC_CACHE = None


def _get_nc():
    global _NC_CACHE
    if _NC_CACHE is None:
        _NC_CACHE = build()
    return _NC_CACHE


def kernel(x, Wq, Wkv, Wo):
    x = np.asarray(x, dtype=np.float32)
    Wq = np.asarray(Wq, dtype=np.float32)
    Wkv = np.asarray(Wkv, dtype=np.float32)
    Wo = np.asarray(Wo, dtype=np.float32)

    # host-side prep (transposes + bf16 casts)
    try:
        import ml_dtypes

        bf16 = ml_dtypes.bfloat16
    except ImportError:  # pragma: no cover
        import jax.numpy as jnp

        bf16 = jnp.bfloat16

    xT_b = [np.ascontiguousarray(x[b].T).astype(bf16) for b in range(B)]

    # multiplicative causal masks for the two diagonal pair positions:
    # pair tile j covers k-tile jdiag = 2*pos + j; element (kl, j*512+ql)
    # is kept iff kl <= ql - 128*jdiag.
    kl = np.arange(128)[:, None]
    ql = np.arange(512)[None, :]
    masks = []
    for pos in range(2):
        cols = []
        for j in range(2):
            jd = 2 * pos + j
            cols.append((kl <= ql - 128 * jd).astype(np.float32))
        masks.append(np.concatenate(cols, axis=1).astype(bf16))
    maskA_np, maskB_np = masks

    ident = np.eye(128, dtype=np.float32).astype(bf16)
    ones = np.ones((128, 1), dtype=np.float32).astype(bf16)

    in_maps = []
    for c in range(NCORES):
        b, j = c // 4, c % 4
        wq_l = Wq[EQ * j:EQ * (j + 1), :]  # [1024, 4096]
        wk_l = Wkv[EKV * j:EKV * (j + 1), :]  # [256, 4096]
        wv_l = Wkv[N_KV * HD + EKV * j:N_KV * HD + EKV * (j + 1), :]
        wall = np.concatenate([wq_l, wk_l, wv_l], axis=0)  # [1536, 4096]
        wallT = np.ascontiguousarray(wall.T).astype(bf16)  # [4096, 1536]
        woT_l = np.ascontiguousarray(Wo[EQ * j:EQ * (j + 1), :].T).astype(bf16)
        in_maps.append(
            {
                "xT": xT_b[b],
                "wallT": wallT,
                "woT": woT_l,
                "maskA": maskA_np,
                "maskB": maskB_np,
                "ident": ident,
                "ones_in": ones,
            }
        )

    nc = _get_nc()
    res = run_bass_kernel_spmd(nc, in_maps, core_ids=list(range(NCORES)))

    out = np.empty((B, T, DIM), dtype=np.float32)
    for b in range(B):
        outT = np.concatenate(
            [res.results[b * 4 + j]["out_part"] for j in range(4)], axis=0
        )  # [4096, 2048]
        out[b] = outT.T
    return out
